# revision 108
# baseline (speedup 1.0000x reference)
"""GPTBigCode MQA attention layer on 8 TRN2 NeuronCores.

Sharding: data-parallel over batch (2) x tensor-parallel over head groups
(4 groups of 4 heads).  Core = (b, g).  Each core computes:
  qkvT = W_qkv[:, cols].T @ X[b].T        (cols = Q cols of group + shared KV)
  per head: scoresT = K^T.T @ Q^T, P = exp(scale*scoresT) (causal, no max-sub),
  attnT = V.T @ P, softmax sums via DVE accumulation of P + one ones-matmul,
  scale by 1/sum, partialT = W_proj[rows].T @ attnT
Host gathers: out[b] = sum_g partialT.T + c_proj_b.

Schedule notes (cost-model driven; 277.9us -> 202.0us modeled, PE 96.5% busy):
  - K/V projection chains run first so attention overlaps remaining Q chains.
  - Causal diagonal k-tiles only touch their live column range (512-128j);
    the boundary mask is a single shared 128x128 triangle.
  - Softmax denominators accumulate P on DVE (bf16) instead of burning a
    full PE pass per k-tile on a ones-matmul; the 128-partition reduction
    runs on the otherwise-idle Pool engine (broadcast result), except the
    final head which uses an all-ones PE matmul to shorten the exposed
    normalization latency chain.
  - c_proj chains of macro m-1 are interleaved between the heads of
    attention(m): attention is Act-bound per k-tile, and these chains are
    the only late-available PE filler.
  - Startup DMAs split across the SP/Act/Pool issue queues so the first
    QKV chain unblocks in ~3us instead of ~21us.
  - V^T -> V transposes go through the DMA XBAR (dma_start_transpose).
  - opool bufs=6: output staging tiles are held until their outT DMA
    completes; 4 slots throttled the c_proj drains (-0.9us).
  - Tail outT DMA issues for odd e-tiles go through the idle Pool queue
    so the Act queue only carries drains (-0.5us); the final chain drains
    in halves on DVE+Act in parallel.
  - NOTE: AluOpType.divide on DVE does NOT lower on real HW (neuronxcc
    rejects it; CoreSim accepts) — normalization must stay recip+mul.

All matmuls bf16 with fp32 PSUM accumulation (validated ~4e-3 scale-rel
error vs fp32 reference).  Everything the device sees is pre-transposed /
pre-sliced on host so every matmul operand is in natural PE layout.
"""

import sys

sys.path.insert(0, "/opt/trn_rl_repo")

import numpy as np
import ml_dtypes

S = 2048
E = 2048
D = 128
HG = 4  # heads per core
SM = 4  # s macro-tiles of 512
ET = 16  # e (contraction) tiles of 128
SCALE = 1.0 / float(np.sqrt(D))  # net softmax input scale (unscale cancels)

_CACHE = {}


def _build_nc():
    import concourse.bacc as bacc
    import concourse.bass_isa as bass_isa
    import concourse.mybir as mybir
    import concourse.tile as tile
    from concourse.bass import ds, ts
    from contextlib import ExitStack

    BF = mybir.dt.bfloat16
    F32 = mybir.dt.float32
    Act = mybir.ActivationFunctionType

    nc = bacc.Bacc("TRN2", target_bir_lowering=False, debug=False)

    xt_d = nc.dram_tensor("xt", (E, S), BF, kind="ExternalInput")
    wq_d = nc.dram_tensor("w_qkv", (E, 768), BF, kind="ExternalInput")
    bq_d = nc.dram_tensor("b_qkv", (768, 1), F32, kind="ExternalInput")
    wp_d = nc.dram_tensor("w_proj", (512, E), BF, kind="ExternalInput")
    tri_d = nc.dram_tensor("tri", (128, 128), BF, kind="ExternalInput")
    out_d = nc.dram_tensor("outT", (E, S), F32, kind="ExternalOutput")

    # QKV output c-tile processing order: K and V first so attention (which
    # needs K, V and successive Q heads) can start while Q chains finish.
    C_ORDER = [4, 5, 0, 1, 2, 3]

    with ExitStack() as ctx:
        tc = ctx.enter_context(tile.TileContext(nc))
        const = ctx.enter_context(tc.tile_pool(name="const", bufs=1))
        xpool = ctx.enter_context(tc.tile_pool(name="xpool", bufs=2))
        ptpool = ctx.enter_context(tc.tile_pool(name="ptpool", bufs=8))
        accpool = ctx.enter_context(tc.tile_pool(name="accpool", bufs=3))
        rpool = ctx.enter_context(tc.tile_pool(name="rpool", bufs=3))
        opool = ctx.enter_context(tc.tile_pool(name="opool", bufs=6))
        psQ = ctx.enter_context(tc.tile_pool(name="psQ", bufs=4, space="PSUM"))
        psSc = ctx.enter_context(tc.tile_pool(name="psSc", bufs=2, space="PSUM"))
        psAt = ctx.enter_context(tc.tile_pool(name="psAt", bufs=2, space="PSUM"))

        # --- persistent SBUF tensors ---
        wq_sb = const.tile([128, ET, 768], BF, tag="wq")  # W_qkv, e-tile major
        wp_sb = const.tile([128, 4, E], BF, tag="wp")  # W_proj, c-tile major
        tri_sb = const.tile([128, 128], BF, tag="tri")  # causal triangle i<=q
        bq_sb = const.tile([128, 6], F32, tag="bq")  # qkv bias per c-tile
        qk_sb = const.tile([128, 5, S], BF, tag="qk")  # Q^T (4 heads) + K^T
        vt_sb = const.tile([128, S], BF, tag="vt")  # V^T staging
        v_sb = const.tile([128, ET, D], BF, tag="v")  # V, k-tile major
        at_sb = const.tile([128, HG, S], BF, tag="at")  # scaled attnT per head
        ones_m = const.tile([128, 128], BF, tag="ones_m")

        # Startup DMAs: interleave wq e-tiles with the m=0 xts e-tiles and
        # split across the SP and Act issue queues, so QKV chain (c0, t)
        # unblocks as soon as both tiles for t have landed instead of after
        # the whole weight load.
        xts0 = xpool.tile([128, ET, 512], BF, tag="xt", name="xts0")
        for t in range(ET):
            eng = (nc.sync, nc.scalar, nc.gpsimd)[t % 3]
            eng.dma_start(out=wq_sb[:, t, :], in_=wq_d[ts(t, 128), :])
            eng.dma_start(out=xts0[:, t, :], in_=xt_d[ts(t, 128), ds(0, 512)])
        # tri/bq/wp are small or late-needed; keep them behind the critical
        # tiles so they neither delay wq/xts nor block QKV drains on Act.
        nc.gpsimd.dma_start(out=tri_sb, in_=tri_d[:, :])
        for c in range(6):
            nc.gpsimd.dma_start(out=bq_sb[:, c : c + 1], in_=bq_d[ts(c, 128), :])
        for c in range(4):
            nc.gpsimd.dma_start(out=wp_sb[:, c, :], in_=wp_d[ts(c, 128), :])
        nc.vector.memset(ones_m, 1.0)

        def proj_chain(mp, eo, tail, pool=None):
            """c_proj partial chain for macro mp, output e-tile eo."""
            smp = ds(mp * 512, 512)
            if pool is None:
                ps_o = psQ.tile([128, 512], F32, tag="mmQ", name="ps_o")
            elif pool is psAt:
                ps_o = pool.tile([128, 512], F32, tag="attnacc", name="ps_o")
            else:
                ps_o = pool.tile([128, 512], F32, tag="sc", name="ps_o")
            for c in range(4):
                nc.tensor.matmul(
                    ps_o,
                    lhsT=wp_sb[:, c, ds(eo * 128, 128)],
                    rhs=at_sb[:, c, smp],
                    start=(c == 0),
                    stop=(c == 3),
                )
            ob = opool.tile([128, 512], F32, tag="ob", name="ob")
            # In the tail there is no more exp work, so alternate full-tile
            # drains between Act and DVE (half-splitting the final tiles
            # costs more in extra DMA descriptors than it saves).
            if tail and eo == ET - 1:
                nc.vector.tensor_copy(out=ob[:, ds(0, 256)], in_=ps_o[:, ds(0, 256)])
                nc.scalar.activation(
                    out=ob[:, ds(256, 256)],
                    in_=ps_o[:, ds(256, 256)],
                    func=Act.Identity,
                    bias=0.0,
                    scale=1.0,
                )
                nc.gpsimd.dma_start(
                    out=out_d[ts(eo, 128), ds(mp * 512, 256)], in_=ob[:, ds(0, 256)]
                )
                nc.scalar.dma_start(
                    out=out_d[ts(eo, 128), ds(mp * 512 + 256, 256)],
                    in_=ob[:, ds(256, 256)],
                )
            elif tail and eo % 2 == 1:
                nc.scalar.activation(
                    out=ob, in_=ps_o, func=Act.Identity, bias=0.0, scale=1.0
                )
                nc.gpsimd.dma_start(out=out_d[ts(eo, 128), smp], in_=ob)
            else:
                nc.vector.tensor_copy(out=ob, in_=ps_o)
                nc.sync.dma_start(out=out_d[ts(eo, 128), smp], in_=ob)

        for m in range(SM):
            sm = ds(m * 512, 512)
            # ---- QKV projection for s-macro m: qkvT[:, m*512:...] ----
            if m == 0:
                xts = xts0
            else:
                xts = xpool.tile([128, ET, 512], BF, tag="xt")
                for t in range(ET):
                    nc.sync.dma_start(out=xts[:, t, :], in_=xt_d[ts(t, 128), sm])
            for c in C_ORDER:
                ps = psQ.tile([128, 512], F32, tag="mmQ")
                for t in range(ET):
                    nc.tensor.matmul(
                        ps,
                        lhsT=wq_sb[:, t, ds(c * 128, 128)],
                        rhs=xts[:, t, :],
                        start=(t == 0),
                        stop=(t == ET - 1),
                    )
                dest = qk_sb[:, c, sm] if c < 5 else vt_sb[:, sm]
                nc.vector.tensor_scalar_add(dest, ps, bq_sb[:, c : c + 1])

            # ---- transpose V^T slices -> V rows via the DMA XBAR ----
            # (no PE pass, no PSUM bank, no DVE drain)
            for j in range(4):
                kt = 4 * m + j
                nc.sync.dma_start_transpose(
                    out=v_sb[:, kt, :], in_=vt_sb[:, ds(kt * 128, 128)]
                )

            # ---- attention for q-macro m, interleaved with c_proj(m-1) ----
            # Diagonal k-tiles are narrowed to their live column range.
            # c_proj chains of the previous macro are spread between heads:
            # attention is Act-bound per tile (exp 612ns vs 426ns of PE
            # work), so these chains backfill the PE during exp stalls; the
            # last macro weights them toward late heads where nothing else
            # is left to fill with.
            nkt = 4 * (m + 1)
            for h in range(HG):
                if m > 0:
                    if m == SM - 1:
                        bounds = [0, 1, 4, 9, 16]
                    else:
                        bounds = [0, 4, 8, 12, 16]
                    for eo in range(bounds[h], bounds[h + 1]):
                        proj_chain(m - 1, eo, False)
                ps_at = psAt.tile([128, 512], F32, tag="attnacc")
                acc = accpool.tile([128, 512], BF, tag="acc")

                # Work units: one per k-tile; diagonal tiles narrowed to the
                # live column range.
                units = list(range(nkt))

                def scores(kt):
                    j = kt - 4 * m
                    off = 0 if j < 0 else j * 128
                    live = 512 - off
                    ps_sc = psSc.tile([128, 512], F32, tag="sc", name="ps_sc")
                    # kt==0: exp writes the accumulator directly (saves a
                    # DVE copy and a dependency hop at every head start).
                    pt = acc if kt == 0 else ptpool.tile(
                        [128, 512], BF, tag="pt", name="pt"
                    )
                    nc.tensor.matmul(
                        ps_sc[:, ds(off, live)],
                        lhsT=qk_sb[:, 4, ds(kt * 128, 128)],
                        rhs=qk_sb[:, h, ds(m * 512 + off, live)],
                        start=True,
                        stop=True,
                    )
                    nc.scalar.activation(
                        out=pt[:, ds(off, live)],
                        in_=ps_sc[:, ds(off, live)],
                        func=Act.Exp,
                        bias=0.0,
                        scale=SCALE,
                    )
                    if j >= 0:
                        nc.vector.tensor_mul(
                            pt[:, ds(off, 128)], pt[:, ds(off, 128)], tri_sb
                        )
                    return pt

                def consume(kt, pt):
                    j = kt - 4 * m
                    off = 0 if j < 0 else j * 128
                    live = 512 - off
                    if kt != 0:
                        # Final head of the last macro: route diag adds to the
                        # idle Pool engine so the DVE queue reaches the
                        # normalization (recip+scale) sooner.
                        if m == SM - 1 and h == HG - 1 and j >= 1:
                            nc.gpsimd.tensor_add(
                                acc[:, ds(off, live)],
                                acc[:, ds(off, live)],
                                pt[:, ds(off, live)],
                            )
                        else:
                            nc.vector.tensor_add(
                                acc[:, ds(off, live)],
                                acc[:, ds(off, live)],
                                pt[:, ds(off, live)],
                            )
                    nc.tensor.matmul(
                        ps_at[:, ds(off, live)],
                        lhsT=v_sb[:, kt, :],
                        rhs=pt[:, ds(off, live)],
                        start=(kt == 0),
                        stop=(kt == nkt - 1),
                    )

                # Software pipeline: scores/exp run two units ahead of the
                # AV matmuls (saturates at two: the scores PSUM ring has
                # two banks).
                pend = [scores(units[0])]
                if len(units) > 1:
                    pend.append(scores(units[1]))
                for i, unit in enumerate(units):
                    pt = pend.pop(0)
                    if i + 2 < len(units):
                        pend.append(scores(units[i + 2]))
                    consume(unit, pt)

                # Partition-reduce the P accumulator on the idle Pool engine
                # (result broadcast to all partitions), freeing both a PSUM
                # bank and 16 PE passes. Exception: the very last head's
                # normalization is latency-exposed (nothing left to backfill
                # the PE), so use the faster all-ones PE matmul there; the
                # scores ring is free at that point.
                if m == SM - 1 and h == HG - 1:
                    ps_sum = psSc.tile([128, 512], F32, tag="sc", name="ps_sum")
                    nc.tensor.matmul(
                        ps_sum, lhsT=ones_m, rhs=acc, start=True, stop=True
                    )
                    recip = rpool.tile([128, 512], F32, tag="recip")
                    nc.vector.reciprocal(recip, ps_sum)
                else:
                    sums = rpool.tile([128, 512], F32, tag="sums", name="sums")
                    nc.gpsimd.partition_all_reduce(
                        sums, acc, 128, bass_isa.ReduceOp.add
                    )
                    recip = rpool.tile([128, 512], F32, tag="recip")
                    nc.vector.reciprocal(recip, sums)
                nc.vector.tensor_mul(at_sb[:, h, sm], ps_at, recip)

        # ---- c_proj for the last macro (pure PE work after the last exp) ----
        # The first chain borrows the attention-accumulator PSUM ring: its
        # h2 bank is already free during the final normalization, so this
        # chain streams c0-c2 while the psQ ring is still draining.
        for eo in range(ET):
            pool = (psAt, psSc, psSc, psAt)[eo] if eo < 4 else None
            proj_chain(SM - 1, eo, True, pool=pool)

    nc.compile()
    return nc


def _get_nc():
    if "nc" not in _CACHE:
        _CACHE["nc"] = _build_nc()
    return _CACHE["nc"]


def _host_tri():
    i = np.arange(128)[:, None]
    q = np.arange(128)[None, :]
    return (i <= q).astype(ml_dtypes.bfloat16)


def _in_maps(inputs):
    hidden = np.asarray(inputs["hidden_states"], dtype=np.float32)
    caw = np.asarray(inputs["c_attn_w"], dtype=np.float32)
    cab = np.asarray(inputs["c_attn_b"], dtype=np.float32)
    cpw = np.asarray(inputs["c_proj_w"], dtype=np.float32)

    bf16 = ml_dtypes.bfloat16
    trib = _host_tri()
    xt_by_batch = [hidden[b].T.astype(bf16) for b in range(2)]
    in_maps = []
    for core in range(8):
        b, g = core % 2, core // 2
        cols = np.r_[g * 512 : (g + 1) * 512, E : E + 2 * D]
        in_maps.append(
            {
                "xt": xt_by_batch[b],
                "w_qkv": caw[:, cols].astype(bf16),
                "b_qkv": cab[cols].reshape(768, 1).astype(np.float32),
                "w_proj": cpw[g * 512 : (g + 1) * 512, :].astype(bf16),
                "tri": trib,
            }
        )
    return in_maps


def kernel(**inputs):
    from concourse.bass_utils import run_bass_kernel_spmd

    cpb = np.asarray(inputs["c_proj_b"], dtype=np.float32)
    in_maps = _in_maps(inputs)

    nc = _get_nc()
    res = run_bass_kernel_spmd(nc, in_maps, core_ids=list(range(8)))
    out = np.zeros((2, S, E), np.float32)
    for core in range(8):
        b = core % 2
        out[b] += res.results[core]["outT"].T
    out += cpb[None, None, :]
    return out
